# revision 15
# baseline (speedup 1.0000x reference)
"""Trainium2 Bass kernel for nn_Classifier0 (quadrant-sum classifier).

reference:
    agg[n, q]  = quadrant sums of x[n] (512x512, quadrants of 256x256)
    w          = g * v[..., 0] / ||v||            [4, 4]
    y          = agg[:, :, None] * w + b_fgl      [N, 4, 4]
    out        = y.reshape(N, 16) @ W_fc.T + b_fc [N, 10]

Algebraic refactor (exact in real arithmetic):
    out[n, c] = sum_q agg[n, q] * A[q, c] + cc[c]
      A[q, c] = sum_j w[q, j] * W_fc[c, 4q + j]         (4 x 10, host, fp64)
      cc[c]   = b_fgl.ravel() @ W_fc[c] + b_fc[c]       (10, host, fp64)

Device work (data-parallel, 32 samples per core, C=8 samples per chunk):

Two of this chip's NeuronCores have one SDMA engine (serving 8 SBUF
partitions) that runs ~20% slower than the other 15 due to periodic
system traffic: NC4's engine 15 (partitions 120-127) and NC2's engine 0
(partitions 0-7).  A uniform [128, F] layout makes those engines the
critical path.  So rows are placed asymmetrically: partitions 0-7 and
120-127 carry 25 rows per chunk, partitions 8-119 carry 25+8 = 33:

  region A (all 128 partitions, 25 rows each):
      partition p = (sample j = p//16, half t = (p%16)//8, i = p%8)
      holds rows [i*25, i*25+25) of that half        -> one [128, 25*512] DMA
  region B (partitions 8..119, 8 rows each):
      partition 8 + 7*h + b (h = 2*j + t) holds rows [200 + 8*b, ..+8)
      of half h                                      -> one [112, 8*512] DMA

DVE tensor_reduce sums left 256 columns, ACT (in-place activation Copy
with accum_out) sums the right 256 -> bufLA/bufRA/bufLB/bufRB [128, NCH].
The quadrant contraction plus the tiny fc is one PSUM accumulation of 5
zero-mask-weighted matmuls; psum [NCH, C*10] row-major equals y [32, 10]
with n = C*k + j.
"""

import numpy as np

N, S = 256, 512
H = S // 2
NCORES = 8
SPC = N // NCORES  # samples per core (32)
NCLS = 10

C = 8  # samples per DMA chunk
NCH = SPC // C  # chunks per core (4)
PPS = 128 // C  # partitions per sample in region A (16)
RA = 25  # region-A rows per partition
RB = 8  # region-B rows per partition
PB0, PB1 = 8, 120  # region-B partition range
NHALF = 2 * C  # halves per chunk (16)
BPH = (S // 2 - (PPS // 2) * RA) // RB  # B partitions per half (7)

_PROGRAM_CACHE = {}


def _build_program():
    from contextlib import ExitStack

    import concourse.bacc as bacc
    import concourse.mybir as mybir
    import concourse.tile as tile

    assert (PPS // 2) * RA + BPH * RB == 256
    assert BPH * NHALF == PB1 - PB0

    nc = bacc.Bacc("TRN2", target_bir_lowering=False, debug=False)
    dt = mybir.dt.float32

    # x as [chunk, sample, half, row, col]
    x_t = nc.dram_tensor("x", [NCH, C, 2, H, S], dt, kind="ExternalInput")
    wA_t = nc.dram_tensor("wa", [128, 2 * C * NCLS], dt, kind="ExternalInput")
    wB_t = nc.dram_tensor("wb", [128, 2 * C * NCLS], dt, kind="ExternalInput")
    ccb_t = nc.dram_tensor("ccbt", [1, C * NCLS], dt, kind="ExternalInput")
    y_t = nc.dram_tensor("y", [SPC, NCLS], dt, kind="ExternalOutput")

    with tile.TileContext(nc) as tc, ExitStack() as ctx:
        apool = ctx.enter_context(tc.tile_pool(name="ap", bufs=2))
        cpool = ctx.enter_context(tc.tile_pool(name="cp", bufs=1))
        ppool = ctx.enter_context(tc.tile_pool(name="pp", bufs=1, space="PSUM"))

        x_ap = x_t.ap()

        bufLA = cpool.tile([128, NCH], dt)
        bufRA = cpool.tile([128, NCH], dt)
        bufLB = cpool.tile([128, NCH], dt)
        bufRB = cpool.tile([128, NCH], dt)

        # Region-B tiles are manually double-buffered persistent tiles:
        # compute APs must start at partition 0, so the B reduces span all
        # 128 partitions, and partitions outside [PB0, PB1) — which the B
        # DMA never writes — are zeroed once here (their zero sums are also
        # killed by zero mask rows in the matmul).  GpSimd is otherwise
        # idle and this overlaps the startup preamble.
        tbs = []
        for name in ("tb0", "tb1"):
            tb = cpool.tile([128, RB * S], dt, tag=name)
            # compute APs must start at partition 0/32/64/96; zero the
            # head/tail blocks (the B DMA later overwrites [PB0, PB1))
            nc.gpsimd.memset(tb[0:32, :], 0.0)
            nc.gpsimd.memset(tb[96:128, :], 0.0)
            tbs.append(tb)

        # constant loads on the idle GpSimd SWDGE queue so the SP queue
        # starts streaming x immediately
        wA = cpool.tile([128, 2 * C * NCLS], dt)
        nc.gpsimd.dma_start(wA[:], wA_t.ap())
        wB = cpool.tile([128, 2 * C * NCLS], dt)
        nc.gpsimd.dma_start(wB[:], wB_t.ap())
        ccbt = cpool.tile([1, C * NCLS], dt)
        nc.gpsimd.dma_start(ccbt[:], ccb_t.ap())
        ones1 = cpool.tile([1, NCH], dt)
        nc.vector.memset(ones1[:], 1.0)

        for k in range(NCH):
            xk = x_ap[k]
            # region B first: small, lands early, reduced while A streams
            tb = tbs[k % 2]
            # dma_start only needs matching total size, so the DRAM-side AP
            # keeps its (sample, half, sub-block) dims un-merged
            in_b = xk[:, :, (PPS // 2) * RA : H, :].rearrange(
                "j t (b r) c -> j t b (r c)", b=BPH
            )
            nc.sync.dma_start(tb[PB0:PB1, :], in_b)

            ta = apool.tile([128, RA * S], dt)
            in_a = xk[:, :, 0 : (PPS // 2) * RA, :].rearrange(
                "j t (i r) c -> j t i (r c)", i=PPS // 2
            )
            nc.sync.dma_start(ta[:], in_a)

            bv = tb[:].rearrange("p (r c) -> p r c", c=S)
            nc.vector.tensor_reduce(
                bufLB[:, k : k + 1],
                bv[:, :, 0:H],
                axis=mybir.AxisListType.XY,
                op=mybir.AluOpType.add,
            )
            nc.scalar.activation(
                bv[:, :, H:S],
                bv[:, :, H:S],
                mybir.ActivationFunctionType.Copy,
                accum_out=bufRB[:, k : k + 1],
            )

            av = ta[:].rearrange("p (r c) -> p r c", c=S)
            nc.vector.tensor_reduce(
                bufLA[:, k : k + 1],
                av[:, :, 0:H],
                axis=mybir.AxisListType.XY,
                op=mybir.AluOpType.add,
            )
            nc.scalar.activation(
                av[:, :, H:S],
                av[:, :, H:S],
                mybir.ActivationFunctionType.Copy,
                accum_out=bufRA[:, k : k + 1],
            )

        psum = ppool.tile([NCH, C * NCLS], dt)
        M = C * NCLS
        nc.tensor.matmul(psum[:], lhsT=bufLA[:], rhs=wA[:, 0:M], start=True, stop=False)
        nc.tensor.matmul(psum[:], lhsT=bufRA[:], rhs=wA[:, M : 2 * M], start=False, stop=False)
        nc.tensor.matmul(psum[:], lhsT=bufLB[:], rhs=wB[:, 0:M], start=False, stop=False)
        nc.tensor.matmul(psum[:], lhsT=bufRB[:], rhs=wB[:, M : 2 * M], start=False, stop=False)
        nc.tensor.matmul(psum[:], lhsT=ones1[:], rhs=ccbt[:], start=False, stop=True)

        out_sb = cpool.tile([NCH, C * NCLS], dt)
        nc.vector.tensor_copy(out_sb[:], psum[:])
        nc.sync.dma_start(y_t.ap().rearrange("(k j) c -> k (j c)", j=C), out_sb[:])

    nc.compile()
    return nc


def _host_params(v, g, b_fgl, W_fc, b_fc):
    """Fold the tiny params into zero-masked wa/wb [2, 128, C*10], cc [1, C*10]."""
    v64 = v.astype(np.float64)
    w = g.astype(np.float64) * (v64[..., 0] / np.linalg.norm(v64, axis=-1))  # [4,4]
    A = np.einsum("qj,cqj->qc", w, W_fc.astype(np.float64).reshape(NCLS, 4, 4))
    cc = b_fgl.astype(np.float64).reshape(-1) @ W_fc.astype(np.float64).T
    cc = cc + b_fc.astype(np.float64)

    # quadrant ids: 0=TL, 1=BL, 2=BR, 3=TR
    # per (sample j, top?) the left-col weight row is A[0] or A[1],
    # right-col weight row A[3] or A[2]
    def masked(grp_of_p, top_of_p, valid):
        wl = np.zeros((128, C * NCLS))
        wr = np.zeros((128, C * NCLS))
        for p in range(128):
            if not valid[p]:
                continue
            j = grp_of_p[p]
            t = top_of_p[p]
            wl[p, j * NCLS : (j + 1) * NCLS] = A[0] if t else A[1]
            wr[p, j * NCLS : (j + 1) * NCLS] = A[3] if t else A[2]
        return wl, wr

    p = np.arange(128)
    # region A: partition p -> sample p//16, top if (p%16) < 8
    wlA, wrA = masked(p // PPS, (p % PPS) < (PPS // 2), np.ones(128, bool))
    # region B: partition PB0 + 7h + b -> half h: sample h//2, top if h%2==0
    h = (p - PB0) // BPH
    validB = (p >= PB0) & (p < PB1)
    wlB, wrB = masked(
        np.clip(h, 0, None) // 2, (np.clip(h, 0, None) % 2) == 0, validB
    )
    wa = np.concatenate([wlA, wrA], axis=1).astype(np.float32)
    wb = np.concatenate([wlB, wrB], axis=1).astype(np.float32)
    ccbt = np.tile(cc, C).reshape(1, C * NCLS).astype(np.float32)
    return np.ascontiguousarray(wa), np.ascontiguousarray(wb), ccbt


def _run(inputs, trace=False):
    from concourse.bass_utils import run_bass_kernel_spmd

    if "nc" not in _PROGRAM_CACHE:
        _PROGRAM_CACHE["nc"] = _build_program()
    nc = _PROGRAM_CACHE["nc"]

    x = np.asarray(inputs["x"], dtype=np.float32)
    wa, wb, ccbt = _host_params(
        np.asarray(inputs["v"], np.float32),
        np.asarray(inputs["g"], np.float32),
        np.asarray(inputs["b_fgl"], np.float32),
        np.asarray(inputs["W_fc"], np.float32),
        np.asarray(inputs["b_fc"], np.float32),
    )

    x_sh = np.ascontiguousarray(x).reshape(NCORES, NCH, C, 2, H, S)
    in_maps = [
        {"x": x_sh[i], "wa": wa, "wb": wb, "ccbt": ccbt} for i in range(NCORES)
    ]
    res = run_bass_kernel_spmd(nc, in_maps, list(range(NCORES)), trace=trace)
    y = np.concatenate([res.results[i]["y"] for i in range(NCORES)], axis=0)
    return y, res.exec_time_ns


def kernel(**inputs) -> np.ndarray:
    y, _ = _run(inputs, trace=False)
    return y


# revision 20
# speedup vs baseline: 1.3097x; 1.3097x over previous
"""Trainium2 Bass kernel for nn_Classifier0 (quadrant-sum classifier).

reference:
    agg[n, q]  = quadrant sums of x[n] (512x512, quadrants of 256x256)
    w          = g * v[..., 0] / ||v||            [4, 4]
    y          = agg[:, :, None] * w + b_fgl      [N, 4, 4]
    out        = y.reshape(N, 16) @ W_fc.T + b_fc [N, 10]

Algebraic refactor (exact in real arithmetic):
    out[n, c] = sum_q agg[n, q] * A[q, c] + cc[c]
      A[q, c] = sum_j w[q, j] * W_fc[c, 4q + j]         (4 x 10, host, fp64)
      cc[c]   = b_fgl.ravel() @ W_fc[c] + b_fc[c]       (10, host, fp64)

Device work (data-parallel, 32 samples per core, C=8 samples per chunk):

Two of this chip's NeuronCores have one SDMA engine (serving 8 SBUF
partitions) that runs ~20% slower than the other 15 due to periodic
system traffic: NC4's engine 15 (partitions 120-127) and NC2's engine 0
(partitions 0-7).  A uniform [128, F] layout makes those engines the
critical path.  So rows are placed asymmetrically: partitions 0-7 and
120-127 carry 25 rows per chunk, partitions 8-119 carry 25+8 = 33:

  region A (all 128 partitions, 25 rows each):
      partition p = (sample j = p//16, half t = (p%16)//8, i = p%8)
      holds rows [i*25, i*25+25) of that half        -> one [128, 25*512] DMA
  region B (partitions 8..119, 8 rows each):
      partition 8 + 7*h + b (h = 2*j + t) holds rows [200 + 8*b, ..+8)
      of half h                                      -> one [112, 8*512] DMA

DVE tensor_reduce sums left 256 columns, ACT (in-place activation Copy
with accum_out) sums the right 256 -> bufLA/bufRA/bufLB/bufRB [128, NCH].
The quadrant contraction plus the tiny fc is one PSUM accumulation of 5
zero-mask-weighted matmuls; psum [NCH, C*10] row-major equals y [32, 10]
with n = C*k + j.
"""

import numpy as np

N, S = 256, 512
H = S // 2
NCORES = 8
SPC = N // NCORES  # samples per core (32)
NCLS = 10

C = 8  # samples per DMA chunk
NCH = SPC // C  # chunks per core (4)
PPS = 128 // C  # partitions per sample in region A (16)
RA = 25  # region-A rows per partition
RB = 8  # region-B rows per partition
PB0, PB1 = 8, 120  # region-B partition range
NHALF = 2 * C  # halves per chunk (16)
BPH = (S // 2 - (PPS // 2) * RA) // RB  # B partitions per half (7)

_PROGRAM_CACHE = {}


def _build_program():
    from contextlib import ExitStack

    import concourse.bacc as bacc
    import concourse.mybir as mybir
    import concourse.tile as tile

    assert (PPS // 2) * RA + BPH * RB == 256
    assert BPH * NHALF == PB1 - PB0

    nc = bacc.Bacc("TRN2", target_bir_lowering=False, debug=False)
    dt = mybir.dt.float32

    # host pre-arranged per-partition slabs (contiguous simple DMAs)
    xa_t = nc.dram_tensor("xa", [NCH, 128, RA * S], dt, kind="ExternalInput")
    xb_t = nc.dram_tensor("xb", [NCH, PB1 - PB0, RB * S], dt, kind="ExternalInput")
    wA_t = nc.dram_tensor("wa", [128, 2 * C * NCLS], dt, kind="ExternalInput")
    wB_t = nc.dram_tensor("wb", [128, 2 * C * NCLS], dt, kind="ExternalInput")
    ccb_t = nc.dram_tensor("ccbt", [1, C * NCLS], dt, kind="ExternalInput")
    y_t = nc.dram_tensor("y", [SPC, NCLS], dt, kind="ExternalOutput")

    with tile.TileContext(nc) as tc, ExitStack() as ctx:
        apool = ctx.enter_context(tc.tile_pool(name="ap", bufs=2))
        cpool = ctx.enter_context(tc.tile_pool(name="cp", bufs=1))
        ppool = ctx.enter_context(tc.tile_pool(name="pp", bufs=1, space="PSUM"))

        xa_ap = xa_t.ap()
        xb_ap = xb_t.ap()

        bufLA = cpool.tile([128, NCH], dt)
        bufRA = cpool.tile([128, NCH], dt)
        bufLB = cpool.tile([128, NCH], dt)
        bufRB = cpool.tile([128, NCH], dt)

        # Region-B tiles are manually double-buffered persistent tiles:
        # compute APs must start at partition 0, so the B reduces span all
        # 128 partitions, and partitions outside [PB0, PB1) — which the B
        # DMA never writes — are zeroed once here (their zero sums are also
        # killed by zero mask rows in the matmul).  GpSimd is otherwise
        # idle and this overlaps the startup preamble.
        tbs = []
        for name in ("tb0", "tb1"):
            tb = cpool.tile([128, RB * S], dt, tag=name)
            # compute APs must start at partition 0/32/64/96; zero the
            # head/tail blocks (the B DMA later overwrites [PB0, PB1))
            nc.gpsimd.memset(tb[0:32, :], 0.0)
            nc.gpsimd.memset(tb[96:128, :], 0.0)
            tbs.append(tb)

        # constant loads on the idle GpSimd SWDGE queue so the SP queue
        # starts streaming x immediately
        wA = cpool.tile([128, 2 * C * NCLS], dt)
        nc.gpsimd.dma_start(wA[:], wA_t.ap())
        wB = cpool.tile([128, 2 * C * NCLS], dt)
        nc.gpsimd.dma_start(wB[:], wB_t.ap())
        ccbt = cpool.tile([1, C * NCLS], dt)
        nc.gpsimd.dma_start(ccbt[:], ccb_t.ap())
        ones1 = cpool.tile([1, NCH], dt)
        nc.vector.memset(ones1[:], 1.0)

        for k in range(NCH):
            # region B first: small, lands early, reduced while A streams
            tb = tbs[k % 2]
            nc.sync.dma_start(tb[PB0:PB1, :], xb_ap[k])

            ta = apool.tile([128, RA * S], dt)
            nc.sync.dma_start(ta[:], xa_ap[k])

            bv = tb[:].rearrange("p (r c) -> p r c", c=S)
            nc.vector.tensor_reduce(
                bufLB[:, k : k + 1],
                bv[:, :, 0:H],
                axis=mybir.AxisListType.XY,
                op=mybir.AluOpType.add,
            )
            nc.scalar.activation(
                bv[:, :, H:S],
                bv[:, :, H:S],
                mybir.ActivationFunctionType.Copy,
                accum_out=bufRB[:, k : k + 1],
            )

            av = ta[:].rearrange("p (r c) -> p r c", c=S)
            nc.vector.tensor_reduce(
                bufLA[:, k : k + 1],
                av[:, :, 0:H],
                axis=mybir.AxisListType.XY,
                op=mybir.AluOpType.add,
            )
            nc.scalar.activation(
                av[:, :, H:S],
                av[:, :, H:S],
                mybir.ActivationFunctionType.Copy,
                accum_out=bufRA[:, k : k + 1],
            )

        psum = ppool.tile([NCH, C * NCLS], dt)
        M = C * NCLS
        nc.tensor.matmul(psum[:], lhsT=bufLA[:], rhs=wA[:, 0:M], start=True, stop=False)
        nc.tensor.matmul(psum[:], lhsT=bufRA[:], rhs=wA[:, M : 2 * M], start=False, stop=False)
        nc.tensor.matmul(psum[:], lhsT=bufLB[:], rhs=wB[:, 0:M], start=False, stop=False)
        nc.tensor.matmul(psum[:], lhsT=bufRB[:], rhs=wB[:, M : 2 * M], start=False, stop=False)
        nc.tensor.matmul(psum[:], lhsT=ones1[:], rhs=ccbt[:], start=False, stop=True)

        out_sb = cpool.tile([NCH, C * NCLS], dt)
        nc.vector.tensor_copy(out_sb[:], psum[:])
        nc.sync.dma_start(y_t.ap().rearrange("(k j) c -> k (j c)", j=C), out_sb[:])

    nc.compile()
    return nc


def _host_params(v, g, b_fgl, W_fc, b_fc):
    """Fold the tiny params into zero-masked wa/wb [2, 128, C*10], cc [1, C*10]."""
    v64 = v.astype(np.float64)
    w = g.astype(np.float64) * (v64[..., 0] / np.linalg.norm(v64, axis=-1))  # [4,4]
    A = np.einsum("qj,cqj->qc", w, W_fc.astype(np.float64).reshape(NCLS, 4, 4))
    cc = b_fgl.astype(np.float64).reshape(-1) @ W_fc.astype(np.float64).T
    cc = cc + b_fc.astype(np.float64)

    # quadrant ids: 0=TL, 1=BL, 2=BR, 3=TR
    # per (sample j, top?) the left-col weight row is A[0] or A[1],
    # right-col weight row A[3] or A[2]
    def masked(grp_of_p, top_of_p, valid):
        wl = np.zeros((128, C * NCLS))
        wr = np.zeros((128, C * NCLS))
        for p in range(128):
            if not valid[p]:
                continue
            j = grp_of_p[p]
            t = top_of_p[p]
            wl[p, j * NCLS : (j + 1) * NCLS] = A[0] if t else A[1]
            wr[p, j * NCLS : (j + 1) * NCLS] = A[3] if t else A[2]
        return wl, wr

    p = np.arange(128)
    # region A: partition p -> sample p//16, top if (p%16) < 8
    wlA, wrA = masked(p // PPS, (p % PPS) < (PPS // 2), np.ones(128, bool))
    # region B: partition PB0 + 7h + b -> half h: sample h//2, top if h%2==0
    h = (p - PB0) // BPH
    validB = (p >= PB0) & (p < PB1)
    wlB, wrB = masked(
        np.clip(h, 0, None) // 2, (np.clip(h, 0, None) % 2) == 0, validB
    )
    wa = np.concatenate([wlA, wrA], axis=1).astype(np.float32)
    wb = np.concatenate([wlB, wrB], axis=1).astype(np.float32)
    ccbt = np.tile(cc, C).reshape(1, C * NCLS).astype(np.float32)
    return np.ascontiguousarray(wa), np.ascontiguousarray(wb), ccbt


def _run(inputs, trace=False):
    from concourse.bass_utils import run_bass_kernel_spmd

    if "nc" not in _PROGRAM_CACHE:
        _PROGRAM_CACHE["nc"] = _build_program()
    nc = _PROGRAM_CACHE["nc"]

    x = np.asarray(inputs["x"], dtype=np.float32)
    wa, wb, ccbt = _host_params(
        np.asarray(inputs["v"], np.float32),
        np.asarray(inputs["g"], np.float32),
        np.asarray(inputs["b_fgl"], np.float32),
        np.asarray(inputs["W_fc"], np.float32),
        np.asarray(inputs["b_fc"], np.float32),
    )

    # host pre-layout into per-partition slabs:
    #   xa[k, 16j+8t+i, :] = rows [i*RA, (i+1)*RA) of half (j, t) of chunk k
    #   xb[k, 7h+b, :]     = rows [200+b*RB, ..+RB) of half h = 2j+t
    xv = np.ascontiguousarray(x).reshape(NCORES, NCH, C, 2, H, S)
    ra_rows = (PPS // 2) * RA
    xa = xv[:, :, :, :, 0:ra_rows, :].reshape(
        NCORES, NCH, C * 2 * (PPS // 2), RA * S
    )
    xb = xv[:, :, :, :, ra_rows:H, :].reshape(NCORES, NCH, NHALF * BPH, RB * S)
    xa = np.ascontiguousarray(xa)
    xb = np.ascontiguousarray(xb)
    in_maps = [
        {"xa": xa[i], "xb": xb[i], "wa": wa, "wb": wb, "ccbt": ccbt}
        for i in range(NCORES)
    ]
    res = run_bass_kernel_spmd(nc, in_maps, list(range(NCORES)), trace=trace)
    y = np.concatenate([res.results[i]["y"] for i in range(NCORES)], axis=0)
    return y, res.exec_time_ns


def kernel(**inputs) -> np.ndarray:
    y, _ = _run(inputs, trace=False)
    return y


# revision 22
# speedup vs baseline: 1.4160x; 1.0811x over previous
"""Trainium2 Bass kernel for nn_Classifier0 (quadrant-sum classifier).

reference:
    agg[n, q]  = quadrant sums of x[n] (512x512, quadrants of 256x256)
    w          = g * v[..., 0] / ||v||            [4, 4]
    y          = agg[:, :, None] * w + b_fgl      [N, 4, 4]
    out        = y.reshape(N, 16) @ W_fc.T + b_fc [N, 10]

Algebraic refactor (exact in real arithmetic):
    out[n, c] = sum_q agg[n, q] * A[q, c] + cc[c]
      A[q, c] = sum_j w[q, j] * W_fc[c, 4q + j]         (4 x 10, host, fp64)
      cc[c]   = b_fgl.ravel() @ W_fc[c] + b_fc[c]       (10, host, fp64)

Device/distribution design (measured on this chip):
  - Each NeuronCore streams x from HBM at ~424 GB/s (SBUF-fabric bound),
    EXCEPT two cores (jax devices 0 and 4 = physical NC4/NC2) where one
    of the 16 SDMA engines runs ~21.5 GB/s instead of ~26.5 due to
    periodic system traffic, capping those cores ~23% lower.  DMA
    descriptors are assigned to engines round-robin by the outer AP dim,
    so per-engine load cannot be shaped by data layout on the slow cores.
  - Therefore the batch is split unevenly: slow cores take 26 samples,
    fast cores 34 (sum 256), which equalizes per-core stream time.
  - One shared SPMD program with NSLOT=17 chunk slots of C=2 samples
    (2 MB per DMA).  Slots 13..16 read their x DRAM row offset from a
    per-core table; slow cores' entries are out-of-bounds, so those DMAs
    are skipped on-device (bounds_check="skip_entire_dma") and their
    output rows are ignored by the host gather.

Per chunk: DVE tensor_reduce sums the left 256 columns of each row,
ACT (in-place activation Copy with accum_out) the right 256 ->
bufL/bufR [128, NSLOT].  Quadrant contraction + the tiny fc = 3
accumulating zero-masked matmuls into PSUM [NSLOT, C*10], whose
row-major layout equals y rows n = C*k + j.
"""

import numpy as np

N, S = 256, 512
H = S // 2
NCORES = 8
NCLS = 10

C = 2  # samples per DMA chunk
NSLOT = 17  # chunk slots in the program
NSTATIC = 13  # slots always executed (26 samples)
PPS = 128 // C  # partitions per sample (64)
RPP = S // PPS  # image rows per partition (8)
FREE = S * RPP  # floats per partition per chunk (4096)

SLOW_DEVS = (0, 4)  # jax devices mapping to the two impaired NeuronCores
CNT_SLOW, CNT_FAST = NSTATIC, NSLOT  # chunks per core

_PROGRAM_CACHE = {}


def _build_program():
    from contextlib import ExitStack

    import concourse.bacc as bacc
    import concourse.bass as bass
    import concourse.mybir as mybir
    import concourse.tile as tile

    nc = bacc.Bacc("TRN2", target_bir_lowering=False, debug=False)
    dt = mybir.dt.float32

    x_t = nc.dram_tensor("x", [NSLOT * 128, FREE], dt, kind="ExternalInput")
    wal_t = nc.dram_tensor("walm", [128, C * NCLS], dt, kind="ExternalInput")
    war_t = nc.dram_tensor("warm", [128, C * NCLS], dt, kind="ExternalInput")
    ccb_t = nc.dram_tensor("ccbt", [1, C * NCLS], dt, kind="ExternalInput")
    off_t = nc.dram_tensor("offs", [1, NSLOT], mybir.dt.uint32, kind="ExternalInput")
    y_t = nc.dram_tensor("y", [NSLOT * C, NCLS], dt, kind="ExternalOutput")

    with tile.TileContext(nc) as tc, ExitStack() as ctx:
        xpool = ctx.enter_context(tc.tile_pool(name="xp", bufs=6))
        cpool = ctx.enter_context(tc.tile_pool(name="cp", bufs=1))
        ppool = ctx.enter_context(tc.tile_pool(name="pp", bufs=1, space="PSUM"))

        x_ap = x_t.ap()

        bufL = cpool.tile([128, NSLOT], dt)
        bufR = cpool.tile([128, NSLOT], dt)
        # constant loads on the idle GpSimd SWDGE queue so the SP queue
        # starts streaming x immediately
        walm = cpool.tile([128, C * NCLS], dt)
        nc.gpsimd.dma_start(walm[:], wal_t.ap())
        warm = cpool.tile([128, C * NCLS], dt)
        nc.gpsimd.dma_start(warm[:], war_t.ap())
        ccbt = cpool.tile([1, C * NCLS], dt)
        nc.gpsimd.dma_start(ccbt[:], ccb_t.ap())
        offs = cpool.tile([1, NSLOT], mybir.dt.uint32)
        nc.gpsimd.dma_start(offs[:], off_t.ap())
        ones1 = cpool.tile([1, NSLOT], dt)
        nc.vector.memset(ones1[:], 1.0)

        for k in range(NSLOT):
            xt = xpool.tile([128, FREE], dt)
            if k < NSTATIC:
                nc.sync.dma_start(xt[:], x_ap[k * 128 : (k + 1) * 128, :])
            else:
                # dynamic row offset from the per-core table; OOB -> skip
                reg = nc.sync.alloc_register(f"xoff{k}")
                nc.sync.reg_load(reg, offs[0:1, k : k + 1])
                off = nc.sync.snap(reg, donate=True)
                nc.sync.dma_start(
                    xt[:],
                    x_ap[bass.ds(off, 128), :],
                    bounds_check="skip_entire_dma",
                )
            xv = xt[:].rearrange("p (r c) -> p r c", c=S)
            nc.vector.tensor_reduce(
                bufL[:, k : k + 1],
                xv[:, :, 0:H],
                axis=mybir.AxisListType.XY,
                op=mybir.AluOpType.add,
            )
            nc.scalar.activation(
                xv[:, :, H:S],
                xv[:, :, H:S],
                mybir.ActivationFunctionType.Copy,
                accum_out=bufR[:, k : k + 1],
            )

        psum = ppool.tile([NSLOT, C * NCLS], dt)
        nc.tensor.matmul(psum[:], lhsT=bufL[:], rhs=walm[:], start=True, stop=False)
        nc.tensor.matmul(psum[:], lhsT=bufR[:], rhs=warm[:], start=False, stop=False)
        nc.tensor.matmul(psum[:], lhsT=ones1[:], rhs=ccbt[:], start=False, stop=True)

        out_sb = cpool.tile([NSLOT, C * NCLS], dt)
        nc.vector.tensor_copy(out_sb[:], psum[:])
        nc.sync.dma_start(y_t.ap().rearrange("(k j) c -> k (j c)", j=C), out_sb[:])

    nc.compile()
    return nc


def _host_params(v, g, b_fgl, W_fc, b_fc):
    """Fold the tiny params into zero-masked walm/warm [128, C*10], cc [1, C*10]."""
    v64 = v.astype(np.float64)
    w = g.astype(np.float64) * (v64[..., 0] / np.linalg.norm(v64, axis=-1))  # [4,4]
    A = np.einsum("qj,cqj->qc", w, W_fc.astype(np.float64).reshape(NCLS, 4, 4))
    cc = b_fgl.astype(np.float64).reshape(-1) @ W_fc.astype(np.float64).T
    cc = cc + b_fc.astype(np.float64)

    # quadrant ids: 0=TL, 1=BL, 2=BR, 3=TR; p % PPS < PPS/2 -> top half rows
    p = np.arange(128)
    top = (p % PPS) < (PPS // 2)
    al_col = np.where(top[:, None], A[0][None, :], A[1][None, :])  # [128, 10]
    ar_col = np.where(top[:, None], A[3][None, :], A[2][None, :])  # [128, 10]
    grp = p // PPS  # sample-within-chunk of partition p
    walm = np.zeros((128, C * NCLS))
    warm = np.zeros((128, C * NCLS))
    for j in range(C):
        sel = grp == j
        walm[sel, j * NCLS : (j + 1) * NCLS] = al_col[sel]
        warm[sel, j * NCLS : (j + 1) * NCLS] = ar_col[sel]
    ccbt = np.tile(cc, C).reshape(1, C * NCLS)
    return (
        np.ascontiguousarray(walm, dtype=np.float32),
        np.ascontiguousarray(warm, dtype=np.float32),
        np.ascontiguousarray(ccbt, dtype=np.float32),
    )


def _core_counts():
    cnt = [CNT_FAST] * NCORES
    for d in SLOW_DEVS:
        cnt[d] = CNT_SLOW
    total = sum(c * C for c in cnt)
    assert total == N, (total, N)
    return cnt


def _run(inputs, trace=False):
    from concourse.bass_utils import run_bass_kernel_spmd

    if "nc" not in _PROGRAM_CACHE:
        _PROGRAM_CACHE["nc"] = _build_program()
    nc = _PROGRAM_CACHE["nc"]

    x = np.ascontiguousarray(np.asarray(inputs["x"], dtype=np.float32))
    walm, warm, ccbt = _host_params(
        np.asarray(inputs["v"], np.float32),
        np.asarray(inputs["g"], np.float32),
        np.asarray(inputs["b_fgl"], np.float32),
        np.asarray(inputs["W_fc"], np.float32),
        np.asarray(inputs["b_fc"], np.float32),
    )

    cnt = _core_counts()
    starts = np.concatenate([[0], np.cumsum([c * C for c in cnt])])
    x_flat = x.reshape(N, 128 // C, FREE)  # sample -> its 64-partition block

    in_maps = []
    for i in range(NCORES):
        xi = np.zeros((NSLOT * 128, FREE), np.float32)
        ns = cnt[i] * C
        shard = x_flat[starts[i] : starts[i] + ns]  # [ns, 64, FREE]
        xi[: ns * (128 // C)] = shard.reshape(ns * (128 // C), FREE)
        offs = np.full((1, NSLOT), NSLOT * 128, np.uint32)  # OOB -> skip
        offs[0, : cnt[i]] = np.arange(cnt[i], dtype=np.uint32) * 128
        in_maps.append(
            {"x": xi, "walm": walm, "warm": warm, "ccbt": ccbt, "offs": offs}
        )

    res = run_bass_kernel_spmd(nc, in_maps, list(range(NCORES)), trace=trace)
    y = np.concatenate(
        [res.results[i]["y"][: cnt[i] * C] for i in range(NCORES)], axis=0
    )
    return y, res.exec_time_ns


def kernel(**inputs) -> np.ndarray:
    y, _ = _run(inputs, trace=False)
    return y


# revision 23
# speedup vs baseline: 1.5163x; 1.0709x over previous
"""Trainium2 Bass kernel for nn_Classifier0 (quadrant-sum classifier).

reference:
    agg[n, q]  = quadrant sums of x[n] (512x512, quadrants of 256x256)
    w          = g * v[..., 0] / ||v||            [4, 4]
    y          = agg[:, :, None] * w + b_fgl      [N, 4, 4]
    out        = y.reshape(N, 16) @ W_fc.T + b_fc [N, 10]

Algebraic refactor (exact in real arithmetic):
    out[n, c] = sum_q agg[n, q] * A[q, c] + cc[c]
      A[q, c] = sum_j w[q, j] * W_fc[c, 4q + j]         (4 x 10, host, fp64)
      cc[c]   = b_fgl.ravel() @ W_fc[c] + b_fc[c]       (10, host, fp64)

Device work (data-parallel, 32 samples per core, C=2 samples per chunk):
  - per chunk: one contiguous 2 MB DMA into a [128, 4096] tile
    (partition p holds 8 consecutive image rows of sample p // 64).
  - DVE tensor_reduce sums the left 256 columns of each row, ACT
    (in-place activation Copy with accum_out) sums the right 256
    -> bufL/bufR [128, 16].
  - quadrant contraction + the tiny fc in one PSUM accumulation of 3
    matmuls with zero-masked weights (mask isolates each of the C
    samples interleaved in the partition dim); psum [16, 20] row-major
    equals y [32, 10] with n = C*k + j.  One copy + one output DMA.

The stream is SBUF-fabric bound: ~26.5 GB/s x 16 SDMA engines
= ~424 GB/s per core, so 33.6 MB streams in ~79 us; with fixed
startup (~7 us) and drain/finish (~10 us) overheads the kernel runs
in ~97 us per core.  (On some executions one SDMA engine of a core is
slowed ~20% by roaming system traffic; descriptors are round-robined
over engines by the outer AP dim, so this cannot be countered by
layout, and it moves between runs so it cannot be countered by uneven
sharding either.)
"""

import numpy as np

N, S = 256, 512
H = S // 2
NCORES = 8
SPC = N // NCORES  # samples per core (32)
NCLS = 10

C = 2  # samples per DMA chunk
NCH = SPC // C  # chunks per core (16)
PPS = 128 // C  # partitions per sample (64)
RPP = S // PPS  # image rows per partition (8)
FREE = S * RPP  # floats per partition per chunk (4096)

_PROGRAM_CACHE = {}


def _build_program():
    from contextlib import ExitStack

    import concourse.bacc as bacc
    import concourse.mybir as mybir
    import concourse.tile as tile

    nc = bacc.Bacc("TRN2", target_bir_lowering=False, debug=False)
    dt = mybir.dt.float32

    x_t = nc.dram_tensor("x", [NCH, 128, FREE], dt, kind="ExternalInput")
    wal_t = nc.dram_tensor("walm", [128, C * NCLS], dt, kind="ExternalInput")
    war_t = nc.dram_tensor("warm", [128, C * NCLS], dt, kind="ExternalInput")
    ccb_t = nc.dram_tensor("ccbt", [1, C * NCLS], dt, kind="ExternalInput")
    y_t = nc.dram_tensor("y", [SPC, NCLS], dt, kind="ExternalOutput")

    with tile.TileContext(nc) as tc, ExitStack() as ctx:
        xpool = ctx.enter_context(tc.tile_pool(name="xp", bufs=6))
        cpool = ctx.enter_context(tc.tile_pool(name="cp", bufs=1))
        ppool = ctx.enter_context(tc.tile_pool(name="pp", bufs=1, space="PSUM"))

        x_ap = x_t.ap()

        bufL = cpool.tile([128, NCH], dt)
        bufR = cpool.tile([128, NCH], dt)
        # constant loads go on the scalar engine's HWDGE ring: the SP ring
        # starts streaming x immediately and GpSimd stays fully idle (its
        # end-of-program DGE drain is the expensive one)
        walm = cpool.tile([128, C * NCLS], dt)
        nc.scalar.dma_start(walm[:], wal_t.ap())
        warm = cpool.tile([128, C * NCLS], dt)
        nc.scalar.dma_start(warm[:], war_t.ap())
        ccbt = cpool.tile([1, C * NCLS], dt)
        nc.scalar.dma_start(ccbt[:], ccb_t.ap())
        ones1 = cpool.tile([1, NCH], dt)
        nc.vector.memset(ones1[:], 1.0)

        for k in range(NCH):
            xt = xpool.tile([128, FREE], dt)
            nc.sync.dma_start(xt[:], x_ap[k])
            xv = xt[:].rearrange("p (r c) -> p r c", c=S)
            nc.vector.tensor_reduce(
                bufL[:, k : k + 1],
                xv[:, :, 0:H],
                axis=mybir.AxisListType.XY,
                op=mybir.AluOpType.add,
            )
            nc.scalar.activation(
                xv[:, :, H:S],
                xv[:, :, H:S],
                mybir.ActivationFunctionType.Copy,
                accum_out=bufR[:, k : k + 1],
            )

        psum = ppool.tile([NCH, C * NCLS], dt)
        nc.tensor.matmul(psum[:], lhsT=bufL[:], rhs=walm[:], start=True, stop=False)
        nc.tensor.matmul(psum[:], lhsT=bufR[:], rhs=warm[:], start=False, stop=False)
        nc.tensor.matmul(psum[:], lhsT=ones1[:], rhs=ccbt[:], start=False, stop=True)

        out_sb = cpool.tile([NCH, C * NCLS], dt)
        nc.vector.tensor_copy(out_sb[:], psum[:])
        nc.sync.dma_start(y_t.ap().rearrange("(k j) c -> k (j c)", j=C), out_sb[:])

    nc.compile()
    return nc


def _host_params(v, g, b_fgl, W_fc, b_fc):
    """Fold the tiny params into zero-masked walm/warm [128, C*10], cc [1, C*10]."""
    v64 = v.astype(np.float64)
    w = g.astype(np.float64) * (v64[..., 0] / np.linalg.norm(v64, axis=-1))  # [4,4]
    A = np.einsum("qj,cqj->qc", w, W_fc.astype(np.float64).reshape(NCLS, 4, 4))
    cc = b_fgl.astype(np.float64).reshape(-1) @ W_fc.astype(np.float64).T
    cc = cc + b_fc.astype(np.float64)

    # quadrant ids: 0=TL, 1=BL, 2=BR, 3=TR; p % PPS < PPS/2 -> top half rows
    p = np.arange(128)
    top = (p % PPS) < (PPS // 2)
    al_col = np.where(top[:, None], A[0][None, :], A[1][None, :])  # [128, 10]
    ar_col = np.where(top[:, None], A[3][None, :], A[2][None, :])  # [128, 10]
    grp = p // PPS  # sample-within-chunk of partition p
    walm = np.zeros((128, C * NCLS))
    warm = np.zeros((128, C * NCLS))
    for j in range(C):
        sel = grp == j
        walm[sel, j * NCLS : (j + 1) * NCLS] = al_col[sel]
        warm[sel, j * NCLS : (j + 1) * NCLS] = ar_col[sel]
    ccbt = np.tile(cc, C).reshape(1, C * NCLS)
    return (
        np.ascontiguousarray(walm, dtype=np.float32),
        np.ascontiguousarray(warm, dtype=np.float32),
        np.ascontiguousarray(ccbt, dtype=np.float32),
    )


def _run(inputs, trace=False):
    from concourse.bass_utils import run_bass_kernel_spmd

    if "nc" not in _PROGRAM_CACHE:
        _PROGRAM_CACHE["nc"] = _build_program()
    nc = _PROGRAM_CACHE["nc"]

    x = np.ascontiguousarray(np.asarray(inputs["x"], dtype=np.float32))
    walm, warm, ccbt = _host_params(
        np.asarray(inputs["v"], np.float32),
        np.asarray(inputs["g"], np.float32),
        np.asarray(inputs["b_fgl"], np.float32),
        np.asarray(inputs["W_fc"], np.float32),
        np.asarray(inputs["b_fc"], np.float32),
    )

    x_sh = x.reshape(NCORES, NCH, 128, FREE)
    in_maps = [
        {"x": x_sh[i], "walm": walm, "warm": warm, "ccbt": ccbt}
        for i in range(NCORES)
    ]
    res = run_bass_kernel_spmd(nc, in_maps, list(range(NCORES)), trace=trace)
    y = np.concatenate([res.results[i]["y"] for i in range(NCORES)], axis=0)
    return y, res.exec_time_ns


def kernel(**inputs) -> np.ndarray:
    y, _ = _run(inputs, trace=False)
    return y


# revision 24
# speedup vs baseline: 1.5388x; 1.0148x over previous
"""Trainium2 Bass kernel for nn_Classifier0 (quadrant-sum classifier).

reference:
    agg[n, q]  = quadrant sums of x[n] (512x512, quadrants of 256x256)
    w          = g * v[..., 0] / ||v||            [4, 4]
    y          = agg[:, :, None] * w + b_fgl      [N, 4, 4]
    out        = y.reshape(N, 16) @ W_fc.T + b_fc [N, 10]

Algebraic refactor (exact in real arithmetic):
    out[n, c] = sum_q agg[n, q] * A[q, c] + cc[c]
      A[q, c] = sum_j w[q, j] * W_fc[c, 4q + j]         (4 x 10, host, fp64)
      cc[c]   = b_fgl.ravel() @ W_fc[c] + b_fc[c]       (10, host, fp64)

Device work (data-parallel, 32 samples per core, 1 sample per chunk):
  - per chunk: one contiguous 1 MB DMA into a [128, 2048] tile
    (partition p holds 4 consecutive image rows; p < 64 is the top half).
  - DVE tensor_reduce sums the left 256 columns of each row, ACT
    (in-place activation Copy with accum_out) sums the right 256
    -> bufL/bufR [128, 32].  The last chunk runs both halves on DVE so
    the tail does not pay ACT's end-of-program drain, which otherwise
    lands between ACTIVATE and its accumulator read.
  - quadrant contraction + the tiny fc = PSUM accumulation of 3 matmuls
    (left weights, right weights, ones x bias row).  Output rows 0..29
    are computed/copied/DMAd early (hidden under the tail of the x
    stream); rows 30..31 finish after the last reduce.

Per-core stream is SBUF-fabric bound: 16 SDMA engines x ~26.5 GB/s
= ~424 GB/s -> 33.6 MB in ~79 us; startup preamble ~7 us and
drain/finish ~7 us put a clean core at ~94 us.  On some executions one
SDMA engine of a core is slowed ~20% by roaming system traffic;
descriptors are round-robined over engines by the outer AP dim, so this
can be countered neither by layout nor (since it moves between runs) by
uneven sharding.
"""

import numpy as np

N, S = 256, 512
H = S // 2
NCORES = 8
SPC = N // NCORES  # samples per core (32)
NCLS = 10
RPP = 4  # image rows per partition
FREE = S * RPP  # floats per partition per chunk (2048)
MSPLIT = 30  # output rows finished early

_PROGRAM_CACHE = {}


def _build_program():
    from contextlib import ExitStack

    import concourse.bacc as bacc
    import concourse.mybir as mybir
    import concourse.tile as tile

    nc = bacc.Bacc("TRN2", target_bir_lowering=False, debug=False)
    dt = mybir.dt.float32

    x_t = nc.dram_tensor("x", [SPC, 128, FREE], dt, kind="ExternalInput")
    wal_t = nc.dram_tensor("walm", [128, NCLS], dt, kind="ExternalInput")
    war_t = nc.dram_tensor("warm", [128, NCLS], dt, kind="ExternalInput")
    ccb_t = nc.dram_tensor("ccbt", [1, NCLS], dt, kind="ExternalInput")
    y_t = nc.dram_tensor("y", [SPC, NCLS], dt, kind="ExternalOutput")

    with tile.TileContext(nc) as tc, ExitStack() as ctx:
        xpool = ctx.enter_context(tc.tile_pool(name="xp", bufs=8))
        cpool = ctx.enter_context(tc.tile_pool(name="cp", bufs=1))
        ppool = ctx.enter_context(tc.tile_pool(name="pp", bufs=1, space="PSUM"))

        x_ap = x_t.ap()
        y_ap = y_t.ap()

        bufL = cpool.tile([128, SPC], dt)
        bufR = cpool.tile([128, SPC], dt)
        # constant loads go on the scalar engine's HWDGE ring: the SP ring
        # starts streaming x immediately and GpSimd stays fully idle
        walm = cpool.tile([128, NCLS], dt)
        nc.scalar.dma_start(walm[:], wal_t.ap())
        warm = cpool.tile([128, NCLS], dt)
        nc.scalar.dma_start(warm[:], war_t.ap())
        ccbt = cpool.tile([1, NCLS], dt)
        nc.scalar.dma_start(ccbt[:], ccb_t.ap())
        ones1 = cpool.tile([1, SPC], dt)
        nc.vector.memset(ones1[:], 1.0)

        for k in range(SPC):
            xt = xpool.tile([128, FREE], dt)
            nc.sync.dma_start(xt[:], x_ap[k])
            xv = xt[:].rearrange("p (r c) -> p r c", c=S)
            nc.vector.tensor_reduce(
                bufL[:, k : k + 1],
                xv[:, :, 0:H],
                axis=mybir.AxisListType.XY,
                op=mybir.AluOpType.add,
            )
            if k < SPC - 1:
                nc.scalar.activation(
                    xv[:, :, H:S],
                    xv[:, :, H:S],
                    mybir.ActivationFunctionType.Copy,
                    accum_out=bufR[:, k : k + 1],
                )
            else:
                nc.vector.tensor_reduce(
                    bufR[:, k : k + 1],
                    xv[:, :, H:S],
                    axis=mybir.AxisListType.XY,
                    op=mybir.AluOpType.add,
                )

        # rows [0, MSPLIT): ready once chunk MSPLIT-1 is reduced — overlaps
        # the tail of the x stream
        psumA = ppool.tile([MSPLIT, NCLS], dt)
        nc.tensor.matmul(psumA[:], lhsT=bufL[:, 0:MSPLIT], rhs=walm[:], start=True, stop=False)
        nc.tensor.matmul(psumA[:], lhsT=bufR[:, 0:MSPLIT], rhs=warm[:], start=False, stop=False)
        nc.tensor.matmul(psumA[:], lhsT=ones1[:, 0:MSPLIT], rhs=ccbt[:], start=False, stop=True)
        outA = cpool.tile([MSPLIT, NCLS], dt)
        nc.vector.tensor_copy(outA[:], psumA[:])
        nc.sync.dma_start(y_ap[0:MSPLIT, :], outA[:])

        # rows [MSPLIT, SPC): the short critical tail
        mb = SPC - MSPLIT
        psumB = ppool.tile([mb, NCLS], dt)
        nc.tensor.matmul(psumB[:], lhsT=bufL[:, MSPLIT:SPC], rhs=walm[:], start=True, stop=False)
        nc.tensor.matmul(psumB[:], lhsT=bufR[:, MSPLIT:SPC], rhs=warm[:], start=False, stop=False)
        nc.tensor.matmul(psumB[:], lhsT=ones1[:, MSPLIT:SPC], rhs=ccbt[:], start=False, stop=True)
        outB = cpool.tile([mb, NCLS], dt)
        nc.vector.tensor_copy(outB[:], psumB[:])
        nc.sync.dma_start(y_ap[MSPLIT:SPC, :], outB[:])

    nc.compile()
    return nc


def _host_params(v, g, b_fgl, W_fc, b_fc):
    """Fold the tiny params into walm/warm [128, 10] and cc [1, 10] (fp64 host)."""
    v64 = v.astype(np.float64)
    w = g.astype(np.float64) * (v64[..., 0] / np.linalg.norm(v64, axis=-1))  # [4,4]
    A = np.einsum("qj,cqj->qc", w, W_fc.astype(np.float64).reshape(NCLS, 4, 4))
    cc = b_fgl.astype(np.float64).reshape(-1) @ W_fc.astype(np.float64).T
    cc = cc + b_fc.astype(np.float64)

    # quadrant ids: 0=TL, 1=BL, 2=BR, 3=TR; partition p < 64 -> top half rows
    top = np.arange(128) < 64
    walm = np.where(top[:, None], A[0][None, :], A[1][None, :])
    warm = np.where(top[:, None], A[3][None, :], A[2][None, :])
    return (
        np.ascontiguousarray(walm, dtype=np.float32),
        np.ascontiguousarray(warm, dtype=np.float32),
        np.ascontiguousarray(cc.reshape(1, NCLS), dtype=np.float32),
    )


def _run(inputs, trace=False):
    from concourse.bass_utils import run_bass_kernel_spmd

    if "nc" not in _PROGRAM_CACHE:
        _PROGRAM_CACHE["nc"] = _build_program()
    nc = _PROGRAM_CACHE["nc"]

    x = np.ascontiguousarray(np.asarray(inputs["x"], dtype=np.float32))
    walm, warm, ccbt = _host_params(
        np.asarray(inputs["v"], np.float32),
        np.asarray(inputs["g"], np.float32),
        np.asarray(inputs["b_fgl"], np.float32),
        np.asarray(inputs["W_fc"], np.float32),
        np.asarray(inputs["b_fc"], np.float32),
    )

    x_sh = x.reshape(NCORES, SPC, 128, FREE)
    in_maps = [
        {"x": x_sh[i], "walm": walm, "warm": warm, "ccbt": ccbt}
        for i in range(NCORES)
    ]
    res = run_bass_kernel_spmd(nc, in_maps, list(range(NCORES)), trace=trace)
    y = np.concatenate([res.results[i]["y"] for i in range(NCORES)], axis=0)
    return y, res.exec_time_ns


def kernel(**inputs) -> np.ndarray:
    y, _ = _run(inputs, trace=False)
    return y
